# revision 14
# baseline (speedup 1.0000x reference)
"""Trainium2 Bass kernel for the GNN attention-head message-passing module.

Contract: kernel(**inputs) takes the FULL (unsharded) numpy inputs and
returns the FULL [N, C_OUT] float32 output, distributing work across 8
NeuronCores internally.

Math (reference):
    fts  = seq @ W_seq.T                      [N, CO]
    f1   = fts @ w_f1 + b_f1 ; f2 = fts @ w_f2 + b_f2
    e_e  = selu(f1[src_e] + f2[dst_e])        per edge
    coef = softmax(e) (global over edges)
    vals[n] = sum_{e:src=n} coef_e*fts[dst_e] + (sum_{e:dst=n} coef_e)*fts[n]
    out  = elu(vals + bias + seq @ W_res.T + b_res)

Device strategy (8 cores, SPMD single program):
  - softmax linearity: accumulate with raw exp weights, divide by the
    global Z = sum(exp) once at the end.
  - nodes are assigned to cores round-robin over the out-degree-sorted
    order; the src-ELL (128-slot x edge-column) layout is a shared
    compile-time constant across cores with ~2% padding.
  - a global node table [R*NSH, 129] fp8 ([fts | f2] per row) is built
    from per-core shards with one AllGather.  The aggregation terms are
    ~1e-4 of the residual-path magnitude here, so fp8 storage costs
    nothing in output accuracy; the residual path stays bf16/f32.
  - src sweep: one indirect-DMA gather of the 129B row per edge column
    (HW supports only [P,1] offset vectors), fused exp(selu(.)) on
    DVE/ACT, per-column scaling, U accumulation via identity-matmul
    PSUM reduction.
  - s_dst (sum of exp into each edge's *dst*) is accumulated during the
    same sweep with NO second gather pass: per column an is_equal-mask
    one-hot lhsT (exp values placed at dst%128) matmul'd against an
    is_equal free-axis mask (1 at dst//128) scatter-adds every edge's
    exp into a [128, 392] all-nodes accumulator in PSUM.
  - one fused ReduceScatter returns each core its own nodes' s values
    plus the global Z (appended slot per shard).
"""

import sys

if "/opt/trn_rl_repo" not in sys.path:
    sys.path.insert(0, "/opt/trn_rl_repo")

import numpy as np
import ml_dtypes

P = 128
LAMBDA = 1.0507009873554805
ALPHA = 1.6732632423543772
LA = LAMBDA * ALPHA
MASK_NEG = -100.0
WTAB = 129  # table row: fts(0:128) | f2(128)


def _ceil_to(x, m):
    return ((x + m - 1) // m) * m


class Plan:
    """Host-side sharding plan + per-core input arrays (all numpy)."""

    def __init__(self, seq, edge_index, W_seq, w_f1, b_f1, w_f2, b_f2, bias,
                 W_res, b_res, R=8, BB=64):
        N, C = seq.shape
        CO = W_seq.shape[0]
        E = edge_index.shape[1]
        assert C % P == 0 and CO <= P
        self.N, self.C, self.CO, self.E, self.R = N, C, CO, E, R
        self.BB = BB

        src = np.asarray(edge_index[0], dtype=np.int64)
        dst = np.asarray(edge_index[1], dtype=np.int64)

        npc = -(-N // R)                       # max nodes per core
        NCH = -(-npc // P)                     # chunks per core
        NSH = NCH * P
        self.NCH, self.NSH = NCH, NSH
        NC2 = R * NSH // P                     # global node columns (392)
        self.NC2 = NC2

        # ---------- src side (out-degree) ----------
        deg = np.bincount(src, minlength=N)
        order = np.argsort(-deg, kind="stable")
        rank_of = np.empty(N, np.int64)
        rank_of[order] = np.arange(N)
        core_of = (rank_of % R).astype(np.int64)
        lslot = (rank_of // R).astype(np.int64)
        degs_sorted = deg[order]
        p_src = [int(degs_sorted[min(k * P * R, N - 1)]) for k in range(NCH)]
        off_src = np.concatenate([[0], np.cumsum(p_src)]).astype(np.int64)
        T_src = int(off_src[-1])
        T_srcP = max(_ceil_to(T_src, BB), BB)
        self.p_src, self.off_src, self.T_src, self.T_srcP = p_src, off_src, T_src, T_srcP
        self.order, self.core_of, self.lslot = order, core_of, lslot

        es = np.argsort(src, kind="stable")
        starts = np.zeros(N + 1, np.int64)
        starts[1:] = np.cumsum(deg)
        epos = np.arange(E) - starts[src[es]]
        n_ = src[es]
        c_ = core_of[n_]
        pp = lslot[n_] % P
        kk = lslot[n_] // P
        t_ = off_src[kk] + epos
        row_ = core_of[dst[es]] * NSH + lslot[dst[es]]

        dst_idx = np.zeros((R, P, T_srcP), np.int32)
        maskm = np.full((R, P, T_srcP), MASK_NEG, ml_dtypes.bfloat16)
        posm = np.zeros((R, P, T_srcP), np.float32)
        posd = np.zeros((R, P, T_srcP), np.float32)
        dst_idx[c_, pp, t_] = row_.astype(np.int32)
        maskm[c_, pp, t_] = 0.0
        posm[c_, pp, t_] = (row_ % P).astype(np.float32)
        posd[c_, pp, t_] = (row_ // P).astype(np.float32)
        self.dst_idx, self.maskm = dst_idx, maskm
        self.posm, self.posd = posm, posd

        # iota helper rows (same for every core)
        self.iotaF = np.broadcast_to(
            np.arange(NC2, dtype=np.float32), (P, NC2)).copy()
        self.iota128 = np.broadcast_to(
            np.arange(P, dtype=np.float32), (P, P)).astype(
                ml_dtypes.bfloat16).copy()

        # ---------- per-core dense inputs ----------
        seq = np.asarray(seq, np.float32)
        self.seqT = np.zeros((R, C, NSH), ml_dtypes.bfloat16)
        self.ncore_nodes = []
        for c in range(R):
            nodes_c = order[c::R]
            self.ncore_nodes.append(nodes_c)
            self.seqT[c, :, : len(nodes_c)] = seq[nodes_c].T.astype(
                ml_dtypes.bfloat16)

        W_seq = np.asarray(W_seq, np.float64)
        W_res = np.asarray(W_res, np.float64)
        w_f1 = np.asarray(w_f1, np.float64).reshape(CO)
        w_f2 = np.asarray(w_f2, np.float64).reshape(CO)
        u1 = W_seq.T @ w_f1
        u2 = W_seq.T @ w_f2
        # wcat columns: [W_seq.T (0:CO) | u2 (CO) | u1 (CO+1) | W_res.T]
        self.wcat = np.concatenate(
            [W_seq.T, u2[:, None], u1[:, None], W_res.T], axis=1
        ).astype(ml_dtypes.bfloat16)            # [C, 2*CO+2]
        self.brow = (np.asarray(bias, np.float32) + np.asarray(b_res, np.float32)
                     ).reshape(1, CO)
        self.b12 = float(np.asarray(b_f1, np.float32) + np.asarray(b_f2, np.float32))

    def in_maps(self):
        maps = []
        for c in range(self.R):
            maps.append({
                "seqT": np.ascontiguousarray(self.seqT[c]),
                "wcat": self.wcat,
                "dsti": np.ascontiguousarray(self.dst_idx[c]),
                "maskm": np.ascontiguousarray(self.maskm[c]),
                "posm": np.ascontiguousarray(self.posm[c]),
                "posd": np.ascontiguousarray(self.posd[c]),
                "iotaF": self.iotaF,
                "iota128": self.iota128,
                "brow": self.brow,
            })
        return maps

    def unshard(self, results):
        out = np.empty((self.N, self.CO), np.float32)
        for c in range(self.R):
            nodes_c = self.ncore_nodes[c]
            out[nodes_c] = results[c]["out"][: len(nodes_c)]
        return out


def _segments(t0, t1, off, nch):
    """Chunk segments (k, lo, hi) covering tile range [t0, t1)."""
    segs = []
    for k in range(nch):
        lo = max(int(off[k]), t0)
        hi = min(int(off[k + 1]), t1)
        if lo < hi:
            segs.append((k, lo, hi))
    return segs


def build_program(plan, debug=False, no_ag=False, no_gather=False, no_scatter=False):
    import concourse.bacc as bacc
    import concourse.bass as bass
    import concourse.mybir as mybir
    import concourse.tile as tile
    from concourse.masks import make_identity

    f32 = mybir.dt.float32
    i32 = mybir.dt.int32
    bf16 = mybir.dt.bfloat16
    fp8 = mybir.dt.float8e4
    Alu = mybir.AluOpType
    Act = mybir.ActivationFunctionType
    Ax = mybir.AxisListType
    IOA = bass.IndirectOffsetOnAxis

    R, C, CO = plan.R, plan.C, plan.CO
    NCH, NSH, NC2 = plan.NCH, plan.NSH, plan.NC2
    BB = plan.BB
    T_srcP = plan.T_srcP
    KC = C // P
    WN = 2 * CO + 2

    nc = bacc.Bacc("TRN2", target_bir_lowering=False, debug=False, num_devices=R)

    seqT = nc.dram_tensor("seqT", [C, NSH], bf16, kind="ExternalInput")
    wcat = nc.dram_tensor("wcat", [C, WN], bf16, kind="ExternalInput")
    dsti = nc.dram_tensor("dsti", [P, T_srcP], i32, kind="ExternalInput")
    maskm = nc.dram_tensor("maskm", [P, T_srcP], bf16, kind="ExternalInput")
    posm = nc.dram_tensor("posm", [P, T_srcP], f32, kind="ExternalInput")
    posd = nc.dram_tensor("posd", [P, T_srcP], f32, kind="ExternalInput")
    iotaF = nc.dram_tensor("iotaF", [P, NC2], f32, kind="ExternalInput")
    iota128 = nc.dram_tensor("iota128", [P, P], bf16, kind="ExternalInput")
    brow = nc.dram_tensor("brow", [1, CO], f32, kind="ExternalInput")
    out_sh = nc.dram_tensor("out", [NSH, CO], f32, kind="ExternalOutput")

    table_sh = nc.dram_tensor("table_sh", [NSH, WTAB], fp8)
    table = nc.dram_tensor("table", [R * NSH, WTAB], fp8)
    s_cmp2 = nc.dram_tensor("s_cmp2", [R * (NSH + 1), 1], f32)
    s_own2 = nc.dram_tensor("s_own2", [NSH + 1, 1], f32)
    if debug:
        tdump = nc.dram_tensor("tdump", [R * NSH, WTAB], fp8,
                               kind="ExternalOutput")
        sdump = nc.dram_tensor("sdump", [R * (NSH + 1), 1], f32,
                               kind="ExternalOutput")
        udump = nc.dram_tensor("udump", [P, NCH * CO], f32,
                               kind="ExternalOutput")
        cdump = nc.dram_tensor("cdump", [P, T_srcP], f32,
                               kind="ExternalOutput")

    groups = [list(range(R))]
    n_sb = T_srcP // BB
    CG = 4
    n_cg = -(-NCH // CG)

    with tile.TileContext(nc) as tc:
        with tc.tile_pool(name="persist", bufs=1) as pp, \
             tc.tile_pool(name="work", bufs=3) as wp, \
             tc.tile_pool(name="gbuf", bufs=3) as gp, \
             tc.tile_pool(name="psumU", bufs=2, space="PSUM") as pup, \
             tc.tile_pool(name="psumS", bufs=1, space="PSUM") as psp:

            # persistent stashes
            tableS = pp.tile([P, NCH, WTAB], fp8)
            US = pp.tile([P, NCH, CO], f32)
            resS = pp.tile([P, NCH, CO], f32)
            tmpF = pp.tile([P, NCH, CO], f32)
            f1S = pp.tile([P, NCH], f32)
            sS = pp.tile([P, NC2], f32)
            sMine3 = pp.tile([P, NCH, 1], f32)
            ident = pp.tile([P, P], bf16)
            biasmat3 = pp.tile([P, 1, CO], f32)
            onesRow = pp.tile([1, P], f32)
            onesR8 = pp.tile([P, R], f32)
            zinvC = pp.tile([P, 1], f32)
            zeroB = pp.tile([P, 1], f32)
            negLA = pp.tile([P, 1], f32)
            wcatS = pp.tile([P, KC, WN], bf16)
            browS = pp.tile([1, CO], f32)
            iotaFS = pp.tile([P, NC2], f32)
            iota128S = pp.tile([P, P], bf16)

            nc.vector.memset(zeroB[:], 0.0)
            nc.vector.memset(negLA[:], -LA)
            make_identity(nc, ident[:])
            nc.vector.memset(onesRow[:], 1.0)
            nc.vector.memset(onesR8[:], 1.0)
            nc.sync.dma_start(out=iotaFS[:], in_=iotaF[:, :])
            nc.sync.dma_start(out=iota128S[:], in_=iota128[:, :])

            # ---------------- phase 1: table shard + res ----------------
            nc.sync.dma_start(
                out=wcatS[:],
                in_=wcat[:, :].rearrange("(k p) w -> p k w", p=P))
            nc.sync.dma_start(out=browS[:], in_=brow[:, :])

            with tc.tile_pool(name="ph1psum", bufs=2, space="PSUM") as p1p:
                for g in range(n_cg):
                    m0 = g * CG
                    m1 = min(m0 + CG, NCH)
                    nch_g = m1 - m0
                    lhs = wp.tile([P, KC, CG * P], bf16, name="lhs", tag="lhs")
                    nc.sync.dma_start(
                        out=lhs[:, :, 0:nch_g * P],
                        in_=seqT[:, m0 * P:m1 * P].rearrange(
                            "(k p) n -> p k n", p=P))
                    for m in range(m0, m1):
                        j0 = (m - m0) * P
                        ps1 = p1p.tile([P, WN], f32)
                        for k in range(KC):
                            nc.tensor.matmul(out=ps1[:], lhsT=lhs[:, k, j0:j0 + P],
                                             rhs=wcatS[:, k, :],
                                             start=(k == 0), stop=(k == KC - 1))
                        nc.vector.tensor_copy(out=tableS[:, m, 0:WTAB],
                                              in_=ps1[:, 0:WTAB])
                        nc.vector.tensor_copy(out=resS[:, m, :],
                                              in_=ps1[:, CO + 2:WN])
                        nc.vector.tensor_copy(out=f1S[:, m:m + 1],
                                              in_=ps1[:, CO + 1:CO + 2])
                # bias broadcast matrix: ones^T @ brow
                psb = p1p.tile([P, CO], f32, bufs=1)
                nc.tensor.matmul(out=psb[:], lhsT=onesRow[:], rhs=browS[:],
                                 start=True, stop=True)
                nc.vector.tensor_copy(out=biasmat3[:, 0, :], in_=psb[:])

            nc.sync.dma_start(
                out=table_sh[:, :].rearrange("(m s) f -> s m f", s=P),
                in_=tableS[:])
            if no_ag:
                for r in range(R):
                    nc.sync.dma_start(out=table[r * NSH:(r + 1) * NSH, :],
                                      in_=table_sh[:, :])
            else:
                nc.gpsimd.collective_compute(
                    "AllGather", Alu.bypass, replica_groups=groups,
                    ins=[table_sh[:, :]], outs=[table[:, :]])
            if debug:
                nc.sync.dma_start(out=tdump[:, :], in_=table[:, :])

            # ---------------- phase 2: src sweep ----------------
            psS = psp.tile([P, NC2], f32)
            psum_tiles = {}
            first_scatter = True
            for b in range(n_sb):
                t0, t1 = b * BB, (b + 1) * BB
                idxT = wp.tile([P, BB], i32, name="idxT", tag="idxT")
                nc.sync.dma_start(out=idxT[:], in_=dsti[:, t0:t1])
                G = gp.tile([P, BB, WTAB], fp8, name="G", tag="G")
                if no_gather:
                    nc.vector.memset(G[:], 0.25)
                else:
                    for j in range(BB):
                        nc.gpsimd.indirect_dma_start(
                            out=G[:, j, :], out_offset=None, in_=table[:, :],
                            in_offset=IOA(ap=idxT[:, j:j + 1], axis=0))

                segs = _segments(t0, t1, plan.off_src, NCH)
                cov = sum(hi - lo for _, lo, hi in segs)
                if cov < t1 - t0:               # pad tiles -> pseudo segment
                    segs = segs + [(-1, t0 + cov, t1)]

                coefT = wp.tile([P, BB], f32, name="coefT", tag="coefT")
                for k, lo, hi in segs:
                    kk = max(k, 0)
                    nc.vector.tensor_scalar(
                        out=coefT[:, lo - t0:hi - t0],
                        in0=G[:, lo - t0:hi - t0, CO],
                        scalar1=f1S[:, kk:kk + 1], scalar2=plan.b12,
                        op0=Alu.add, op1=Alu.add)
                mmT = wp.tile([P, BB], bf16, name="mmT", tag="mmT")
                nc.sync.dma_start(out=mmT[:], in_=maskm[:, t0:t1])
                pmT = wp.tile([P, BB], f32, name="pmT", tag="pmT")
                nc.sync.dma_start(out=pmT[:], in_=posm[:, t0:t1])
                pdT = wp.tile([P, BB], f32, name="pdT", tag="pdT")
                nc.sync.dma_start(out=pdT[:], in_=posd[:, t0:t1])
                rT = wp.tile([P, BB], f32, name="rT", tag="rT")
                mT = wp.tile([P, BB], f32, name="mT", tag="mT")
                nc.vector.tensor_scalar_max(out=rT[:], in0=coefT[:], scalar1=0.0)
                nc.vector.tensor_tensor(out=mT[:], in0=coefT[:], in1=rT[:],
                                        op=Alu.subtract)
                nc.vector.tensor_tensor(out=rT[:], in0=rT[:], in1=mmT[:],
                                        op=Alu.add)
                nc.scalar.activation(out=mT[:], in_=mT[:], func=Act.Exp,
                                     bias=zeroB[:])
                nc.scalar.activation(out=mT[:], in_=mT[:], func=Act.Exp,
                                     bias=negLA[:], scale=LA)
                nc.scalar.activation(out=rT[:], in_=rT[:], func=Act.Exp,
                                     bias=zeroB[:], scale=LAMBDA)
                nc.vector.tensor_tensor(out=coefT[:], in0=mT[:], in1=rT[:],
                                        op=Alu.mult)
                if debug:
                    nc.sync.dma_start(out=cdump[:, t0:t1], in_=coefT[:])

                Gs = gp.tile([P, BB, CO], bf16, name="Gs", tag="Gs")
                ohm = wp.tile([P, P], bf16, name="ohm", tag="ohm")
                msk = wp.tile([P, NC2], bf16, name="msk", tag="msk")
                for k, lo, hi in segs:
                    for t in range(lo, hi):
                        j = t - t0
                        # s-scatter: exp one-hot (dst%128) x mask (dst//128)
                        if no_scatter and not (first_scatter or t == T_srcP - 1):
                            pass
                        else:
                            nc.vector.tensor_scalar(
                                out=ohm[:], in0=iota128S[:],
                                scalar1=pmT[:, j:j + 1], scalar2=coefT[:, j:j + 1],
                                op0=Alu.is_equal, op1=Alu.mult)
                            nc.vector.tensor_scalar(
                                out=msk[:], in0=iotaFS[:],
                                scalar1=pdT[:, j:j + 1], scalar2=None,
                                op0=Alu.is_equal)
                            nc.tensor.matmul(out=psS[:], lhsT=ohm[:], rhs=msk[:],
                                             start=first_scatter, stop=(t == T_srcP - 1),
                                             skip_group_check=True)
                            first_scatter = False
                        if k < 0:
                            continue
                        # U accumulation
                        nc.vector.tensor_scalar_mul(
                            out=Gs[:, j, :], in0=G[:, j, 0:CO],
                            scalar1=coefT[:, j:j + 1])
                        if t == plan.off_src[k]:
                            psum_tiles[k] = pup.tile([P, CO], f32, name="psU",
                                                     tag="psU")
                        last = (t == plan.off_src[k + 1] - 1)
                        nc.tensor.matmul(out=psum_tiles[k][:], lhsT=ident[:],
                                         rhs=Gs[:, j, :],
                                         start=(t == plan.off_src[k]), stop=last,
                                         skip_group_check=True)
                        if last:
                            nc.vector.tensor_copy(out=US[:, k, :],
                                                  in_=psum_tiles[k][:])
                            del psum_tiles[k]
            for k in range(NCH):
                if plan.p_src[k] == 0:
                    nc.vector.memset(US[:, k, :], 0.0)

            # ---------------- phase 3: s + Z reduce-scatter ----------------
            nc.vector.tensor_copy(out=sS[:], in_=psS[:])
            with tc.tile_pool(name="zpsum", bufs=2, space="PSUM") as zp:
                zpart = wp.tile([P, 1], f32)
                nc.vector.tensor_reduce(out=zpart[:], in_=sS[:], axis=Ax.X,
                                        op=Alu.add)
                psz = zp.tile([1, R], f32)
                nc.tensor.matmul(out=psz[:], lhsT=zpart[:], rhs=onesR8[:],
                                 start=True, stop=True)
                zrow = wp.tile([1, R], f32)
                nc.vector.tensor_copy(out=zrow[:], in_=psz[:])
            for r in range(R):
                nc.sync.dma_start(
                    out=s_cmp2[r * (NSH + 1):r * (NSH + 1) + NSH, :].rearrange(
                        "(m p) o -> p (m o)", p=P),
                    in_=sS[:, r * NCH:(r + 1) * NCH])
            nc.sync.dma_start(
                out=s_cmp2[:, :].rearrange("(r n) o -> n (r o)", n=NSH + 1)[
                    NSH:NSH + 1, :],
                in_=zrow[:])
            if debug:
                nc.sync.dma_start(out=sdump[:, :], in_=s_cmp2[:, :])
                nc.sync.dma_start(
                    out=udump[:, :].rearrange("p (c f) -> p c f", c=NCH),
                    in_=US[:])
            nc.gpsimd.collective_compute(
                "ReduceScatter", Alu.add, replica_groups=groups,
                ins=[s_cmp2[:, :]], outs=[s_own2[:, :]])

            # ---------------- phase 4: combine + elu ----------------
            with tc.tile_pool(name="z2psum", bufs=2, space="PSUM") as zp2:
                zsb = wp.tile([1, 1], f32)
                nc.sync.dma_start(out=zsb[:], in_=s_own2[NSH:NSH + 1, :])
                zinv1 = wp.tile([1, 1], f32)
                nc.vector.reciprocal(out=zinv1[:], in_=zsb[:])
                psb2 = zp2.tile([P, 1], f32)
                nc.tensor.matmul(out=psb2[:], lhsT=onesRow[:], rhs=zinv1[:],
                                 start=True, stop=True)
                nc.vector.tensor_copy(out=zinvC[:], in_=psb2[:])

            nc.sync.dma_start(
                out=sMine3[:, :, 0],
                in_=s_own2[0:NSH, :].rearrange("(m p) o -> p (m o)", p=P))
            nc.vector.tensor_scalar(out=sMine3[:, :, 0], in0=sMine3[:, :, 0],
                                    scalar1=zinvC[:, 0:1], scalar2=None,
                                    op0=Alu.mult)

            # final = US/Z + fts*(s/Z) + res + bias, then elu
            t_in, s_in = bass.broadcast_tensor_aps(tableS[:, :, 0:CO],
                                                   sMine3[:, :, :])
            nc.vector.tensor_tensor(out=tmpF[:], in0=t_in, in1=s_in, op=Alu.mult)
            nc.vector.tensor_scalar(
                out=US[:, :, :], in0=US[:, :, :],
                scalar1=zinvC[:, 0:1], scalar2=None, op0=Alu.mult)
            nc.vector.tensor_tensor(out=US[:], in0=US[:], in1=tmpF[:], op=Alu.add)
            nc.vector.tensor_tensor(out=US[:], in0=US[:], in1=resS[:], op=Alu.add)
            u_in, b_in = bass.broadcast_tensor_aps(US[:, :, :],
                                                   biasmat3[:, :, :])
            nc.vector.tensor_tensor(out=US[:], in0=u_in, in1=b_in, op=Alu.add)

            for g in range(n_cg):
                m0 = g * CG
                m1 = min(m0 + CG, NCH)
                ng = m1 - m0
                r2 = wp.tile([P, CG, CO], f32, name="r2", tag="r2")
                m2 = wp.tile([P, CG, CO], f32, name="m2", tag="m2")
                nc.vector.tensor_scalar_max(out=r2[:, 0:ng, :],
                                            in0=US[:, m0:m1, :], scalar1=0.0)
                nc.vector.tensor_tensor(out=m2[:, 0:ng, :], in0=US[:, m0:m1, :],
                                        in1=r2[:, 0:ng, :], op=Alu.subtract)
                nc.scalar.activation(out=m2[:, 0:ng, :], in_=m2[:, 0:ng, :],
                                     func=Act.Exp, bias=zeroB[:])
                nc.vector.tensor_tensor(out=m2[:, 0:ng, :], in0=m2[:, 0:ng, :],
                                        in1=r2[:, 0:ng, :], op=Alu.add)
                nc.vector.tensor_scalar(out=m2[:, 0:ng, :], in0=m2[:, 0:ng, :],
                                        scalar1=-1.0, scalar2=None, op0=Alu.add)
                nc.sync.dma_start(
                    out=out_sh[m0 * P:m1 * P, :].rearrange(
                        "(m s) f -> s m f", s=P),
                    in_=m2[:, 0:ng, :])

    nc.compile()
    return nc


def prepare(**inputs):
    """Build plan + program. Returns (plan, nc, in_maps)."""
    plan = Plan(
        np.asarray(inputs["seq"]), np.asarray(inputs["edge_index"]),
        np.asarray(inputs["W_seq"]), np.asarray(inputs["w_f1"]),
        np.asarray(inputs["b_f1"]), np.asarray(inputs["w_f2"]),
        np.asarray(inputs["b_f2"]), np.asarray(inputs["bias"]),
        np.asarray(inputs["W_res"]), np.asarray(inputs["b_res"]))
    nc = build_program(plan)
    return plan, nc, plan.in_maps()


def kernel(**inputs):
    from concourse.bass_utils import run_bass_kernel_spmd
    plan, nc, in_maps = prepare(**inputs)
    res = run_bass_kernel_spmd(nc, in_maps, core_ids=list(range(plan.R)))
    return plan.unshard(res.results)
